# revision 9
# baseline (speedup 1.0000x reference)
"""CP-decomposition loss kernel for Trainium2 (8 NeuronCores, data parallel).

loss = sum_n (sum_r U0[i_n,r]*U1[j_n,r]*U2[k_n,r] - y_n)^2

Strategy (sharding_hint: data-parallel over the 2M observations, tables
replicated): each core processes 250k observations. Gathers use the MoE
dma_gather primitive: tables are padded to 64B rows and packed 4 rows per
256B block so block ids fit int16; the needed row is selected on the DVE
with 4 masks from s = i % 4. Rank-reduction + squared-error on DVE, final
scalar via a ones-matmul on the PE. Host sums the 8 per-core partials.
"""
import numpy as np

RANK = 10
DIM = 100000
N_OBS = 2000000
N_CORES = 8
NC_OBS = N_OBS // N_CORES          # 250000
NBLK = 2048                        # observations per gather block
NBLOCKS = -(-NC_OBS // NBLK)       # 123
NC_PAD = NBLOCKS * NBLK            # 253952
C = NBLK // 128                    # 32 free-dim slots
VB = DIM // 4 + 1                  # 25001 table blocks (last = zeros pad)
IDXF = NBLK // 16                  # 256 int16 per partition per block

_cache = {}


def _build():
    import concourse.bacc as bacc
    import concourse.bass as bass
    import concourse.mybir as mybir
    from concourse.tile import TileContext

    fp32 = mybir.dt.float32
    nc = bacc.Bacc(target_bir_lowering=False, num_swdge_queues=4)

    t4 = [nc.dram_tensor(f"t4_{t}", [VB, 64], fp32, kind="ExternalInput")
          for t in range(3)]
    bidx = [nc.dram_tensor(f"b16_{t}", [NBLOCKS, 128, IDXF], mybir.dt.int16,
                           kind="ExternalInput") for t in range(3)]
    sval = [nc.dram_tensor(f"s_{t}", [NBLOCKS, 128, C], fp32,
                           kind="ExternalInput") for t in range(3)]
    yv = nc.dram_tensor("yv", [NBLOCKS, 128, C], fp32, kind="ExternalInput")
    loss = nc.dram_tensor("loss", [1, 1], fp32, kind="ExternalOutput")

    with TileContext(nc) as tc:
        with tc.tile_pool(name="gp", bufs=8) as gp, \
             tc.tile_pool(name="ip", bufs=20) as ip, \
             tc.tile_pool(name="sp", bufs=10) as sp, \
             tc.tile_pool(name="op", bufs=6) as op, \
             tc.tile_pool(name="fix", bufs=1) as fix, \
             tc.tile_pool(name="ps", bufs=1, space="PSUM") as ps:
            acc = fix.tile([128, C], fp32)
            nc.vector.memset(acc[:], 0.0)
            # j4[p, j] = j  (constant for the 4-way subrow masks)
            j4 = fix.tile([128, 4], fp32)
            for j in range(4):
                nc.vector.memset(j4[:, j:j + 1], float(j))
            j4b = bass.AP(j4.tensor, j4[:].offset,
                          [j4[:].ap[0], [0, C], [1, 4]])
            for b in range(NBLOCKS):
                otiles = []
                for t in range(3):
                    it = ip.tile([128, IDXF], mybir.dt.int16, tag="idx")
                    nc.sync.dma_start(out=it[:], in_=bidx[t][b])
                    g = gp.tile([128, C * 64], fp32, tag=f"g{t}")
                    nc.gpsimd.dma_gather(
                        out_ap=g[:].rearrange("p (c e) -> p c e", e=64),
                        in_ap=t4[t][:],
                        idxs_ap=it[:],
                        num_idxs=NBLK,
                        num_idxs_reg=NBLK,
                        elem_size=64,
                        single_packet=False,
                        queue_num=(3 * b + t) % 4,
                    )
                    st = sp.tile([128, C], fp32, tag="s")
                    nc.sync.dma_start(out=st[:], in_=sval[t][b])
                    # masks M[p, c, j] = (s[p, c] == j)
                    m4 = op.tile([128, C * 4], fp32, tag="m4")
                    stb = bass.AP(st.tensor, st[:].offset,
                                  [st[:].ap[0], [1, C], [0, 4]])
                    nc.vector.tensor_tensor(
                        out=m4[:].rearrange("p (c j) -> p c j", j=4),
                        in0=stb, in1=j4b, op=mybir.AluOpType.is_equal)
                    # masked product over the 4 subrow candidates, j innermost
                    gj = bass.AP(g.tensor, g[:].offset,
                                 [g[:].ap[0], [64, C], [1, RANK], [16, 4]])
                    mj = bass.AP(m4.tensor, m4[:].offset,
                                 [m4[:].ap[0], [4, C], [0, RANK], [1, 4]])
                    tmp = op.tile([128, C * RANK * 4], fp32, tag="tmp")
                    nc.vector.tensor_tensor(
                        out=tmp[:].rearrange("p (c r j) -> p c r j", r=RANK, j=4),
                        in0=gj, in1=mj, op=mybir.AluOpType.mult)
                    ot = op.tile([128, C * RANK], fp32, tag=f"o{t}")
                    nc.vector.tensor_reduce(
                        out=ot[:],
                        in_=tmp[:].rearrange("p (c r j) -> p c r j", r=RANK, j=4),
                        axis=mybir.AxisListType.X, op=mybir.AluOpType.add)
                    otiles.append(ot)
                prod = op.tile([128, C * RANK], fp32, tag="prod")
                nc.vector.tensor_mul(out=prod[:], in0=otiles[0][:], in1=otiles[1][:])
                nc.vector.tensor_mul(out=prod[:], in0=prod[:], in1=otiles[2][:])
                pred = op.tile([128, C], fp32, tag="pred")
                nc.vector.tensor_reduce(
                    out=pred[:],
                    in_=prod[:].rearrange("p (c r) -> p c r", r=RANK),
                    axis=mybir.AxisListType.X, op=mybir.AluOpType.add)
                yt = sp.tile([128, C], fp32, tag="y")
                nc.sync.dma_start(out=yt[:], in_=yv[b])
                d = op.tile([128, C], fp32, tag="d")
                nc.vector.tensor_tensor(out=d[:], in0=pred[:], in1=yt[:],
                                        op=mybir.AluOpType.subtract)
                nc.vector.tensor_mul(out=d[:], in0=d[:], in1=d[:])
                nc.vector.tensor_add(out=acc[:], in0=acc[:], in1=d[:])
            accr = fix.tile([128, 1], fp32)
            nc.vector.tensor_reduce(out=accr[:], in_=acc[:],
                                    axis=mybir.AxisListType.X,
                                    op=mybir.AluOpType.add)
            ones = fix.tile([128, 1], fp32)
            nc.vector.memset(ones[:], 1.0)
            pt = ps.tile([1, 1], fp32, space="PSUM")
            nc.tensor.matmul(out=pt[:], lhsT=accr[:], rhs=ones[:],
                             start=True, stop=True)
            res = fix.tile([1, 1], fp32)
            nc.vector.tensor_copy(out=res[:], in_=pt[:])
            nc.sync.dma_start(out=loss[:], in_=res[:])
    nc.compile()
    return nc


def _prep_table(u):
    t4 = np.zeros((VB, 64), dtype=np.float32)
    v = t4[:DIM // 4].reshape(DIM // 4, 4, 16)
    v[:, :, :RANK] = np.asarray(u, dtype=np.float32).reshape(DIM // 4, 4, RANK)
    return t4


def kernel(indices, y, U0, U1, U2):
    from concourse.bass_utils import run_bass_kernel_spmd

    if "nc" not in _cache:
        _cache["nc"] = _build()
    nc = _cache["nc"]

    indices = np.asarray(indices)
    y = np.asarray(y, dtype=np.float32)
    t4s = [_prep_table(u) for u in (U0, U1, U2)]

    in_maps = []
    for c in range(N_CORES):
        sl = slice(c * NC_OBS, (c + 1) * NC_OBS)
        m = {f"t4_{t}": t4s[t] for t in range(3)}
        for t in range(3):
            it = np.asarray(indices[sl, t], dtype=np.int64)
            b = (it >> 2).astype(np.int16)
            s = (it & 3).astype(np.float32)
            b_pad = np.full(NC_PAD, DIM // 4, dtype=np.int16)
            b_pad[:NC_OBS] = b
            s_pad = np.zeros(NC_PAD, dtype=np.float32)
            s_pad[:NC_OBS] = s
            # wrapped int16 layout: position m -> (partition m%16, free m//16),
            # replicated across the 8 16-partition groups
            w = b_pad.reshape(NBLOCKS, IDXF, 16).transpose(0, 2, 1)
            m[f"b16_{t}"] = np.tile(w, (1, 8, 1)).copy()
            # slot layout: obs m of a block sits at (partition m%128, free m//128)
            m[f"s_{t}"] = s_pad.reshape(NBLOCKS, C, 128).transpose(0, 2, 1).copy()
        y_pad = np.zeros(NC_PAD, dtype=np.float32)
        y_pad[:NC_OBS] = y[sl]
        m["yv"] = y_pad.reshape(NBLOCKS, C, 128).transpose(0, 2, 1).copy()
        in_maps.append(m)

    global _last_in_maps
    _last_in_maps = in_maps
    res = run_bass_kernel_spmd(nc, in_maps, core_ids=list(range(N_CORES)))
    total = np.float32(0.0)
    for c in range(N_CORES):
        total += res.results[c]["loss"][0, 0]
    return np.float32(total)
